# revision 18
# baseline (speedup 1.0000x reference)
"""Distributed Trainium2 Bass kernel for block-causal multi-head attention.

Problem: LayerNorm -> QKV projection -> 8-head attention with block-causal mask
(8 frames x 256 patches) -> output projection + bias.  x: [2, 2048, 512] f32.

Sharding (8 cores): core c handles batch b = c%2 and head-pair hp = c//2
(heads 2hp, 2hp+1).

Key structure (v2 - no PE transposes, fat matmuls, ACT-exp-bound):
  - host ships x BOTH token-major (stats) and transposed xT (projections),
    both bf16; gamma and 1/sqrt(d) folded into weights; weight column-sums
    shipped for the LN mean correction
  - LN: bn_stats on token-major x; rstd = Exp(-0.5*Ln(var+eps)) (one ACT
    table set shared with softmax exp); mean correction enters each
    projection as a rank-1 matmul seed (-mu ganger colsum_w) accumulated into
    the same PSUM as the raw projection
  - Q/K projected feature-major (rhs = xT, N=512 matmuls); V token-major
    (lhsT = xT chunks); rstd applied via one broadcast multiply (Q),
    per-partition tensor_scalar (V), and the softmax exp's per-partition
    scale operand (K - free)
  - scores per (key-block, head) at N=512 into a 2-bank PSUM tile; ONE exp
    per (kb, h) covering only visible elements; AV accumulates per 512-query
    chunk with a ones-column denominator row (M=65); block-causal blocks
    skipped at chunk granularity, odd-frame edges via N=256 matmuls + e=0
    memsets
  - denominator: DRAM-bounce broadcast, fast reciprocal, normalize into
    attn_nm bf16; out-projection computed TRANSPOSED (out^T = w_out^T @
    attn^T, lhsT = w_out slices) -> outT [C, T] bf16 partials; host
    transposes, sums the 4 head-pair partials per batch and adds b_out
"""

import numpy as np
import ml_dtypes

B = 2
T = 2048
C = 512
H = 8
D = 64
INNER = H * D  # 512
NP = 256  # patches per frame
F = 8  # frames
EPS = 1e-5
N_CORES = 8
NT = T // 128  # 16 token tiles of 128
TC = T // 512  # 4 token chunks of 512
CCH = C // 128  # 4 channel chunks of 128

_CACHE = {}


def _build(add_bias: bool, debug_dump: bool = False):
    import concourse.bass as bass
    import concourse.tile as tile
    from concourse import bacc, mybir

    f32 = mybir.dt.float32
    bf16 = mybir.dt.bfloat16
    i32 = mybir.dt.int32
    AF = mybir.ActivationFunctionType
    OP = mybir.AluOpType

    from concourse.tile_rust import add_dep_helper

    nc = bacc.Bacc("TRN2", target_bir_lowering=False, debug=False, num_devices=N_CORES)

    x = nc.dram_tensor("x", [T, C], bf16, kind="ExternalInput")
    xT = nc.dram_tensor("xT", [C, T], bf16, kind="ExternalInput")
    wq = nc.dram_tensor("wq", [C, 2 * D], bf16, kind="ExternalInput")
    wk = nc.dram_tensor("wk", [C, 2 * D], bf16, kind="ExternalInput")
    wv = nc.dram_tensor("wv", [C, 2 * D], bf16, kind="ExternalInput")
    csq = nc.dram_tensor("csq", [1, 2 * D], bf16, kind="ExternalInput")
    csk = nc.dram_tensor("csk", [1, 2 * D], bf16, kind="ExternalInput")
    csv = nc.dram_tensor("csv", [1, 2 * D], bf16, kind="ExternalInput")
    w_out = nc.dram_tensor("w_out", [2 * D, C], bf16, kind="ExternalInput")
    if add_bias:
        qb = nc.dram_tensor("qb", [2 * D, 1], f32, kind="ExternalInput")
        kb_b = nc.dram_tensor("kb", [2 * D, 1], f32, kind="ExternalInput")
        vb = nc.dram_tensor("vb", [1, 2 * D], f32, kind="ExternalInput")
    out = nc.dram_tensor("out", [C, T], bf16, kind="ExternalOutput")
    if debug_dump:
        dbg_rstd = nc.dram_tensor("dbg_rstd", [128, NT], f32, kind="ExternalOutput")
        dbg_q = nc.dram_tensor("dbg_q", [128, T], f32, kind="ExternalOutput")
        dbg_k = nc.dram_tensor("dbg_k", [128, T], f32, kind="ExternalOutput")
        dbg_v = nc.dram_tensor("dbg_v", [128, NT * 2 * (D + 1)], f32, kind="ExternalOutput")
        dbg_a = nc.dram_tensor("dbg_a", [D, 2 * T], f32, kind="ExternalOutput")
        dbg_rec = nc.dram_tensor("dbg_rec", [8, 512], f32, kind="ExternalOutput")

    with tile.TileContext(nc) as tc:
        import contextlib

        with contextlib.ExitStack() as ctx:
            singles = ctx.enter_context(tc.tile_pool(name="singles", bufs=1))
            work = ctx.enter_context(tc.tile_pool(name="work", bufs=2))
            epool = ctx.enter_context(tc.tile_pool(name="epool", bufs=3))
            epi = ctx.enter_context(tc.tile_pool(name="epi", bufs=2))
            ps_proj = ctx.enter_context(tc.tile_pool(name="ps_proj", bufs=2, space="PSUM"))
            ps_sc = ctx.enter_context(tc.tile_pool(name="ps_sc", bufs=1, space="PSUM"))
            ps_av = ctx.enter_context(tc.tile_pool(name="ps_av", bufs=4, space="PSUM"))
            dram = ctx.enter_context(tc.tile_pool(name="dram", bufs=1, space="DRAM"))

            # ---------------- weights (gpsimd queue) ----------------
            wq_sb = singles.tile([128, CCH, 2 * D], bf16)
            nc.gpsimd.dma_start(wq_sb[:], wq.ap().rearrange("(cc p) d -> p cc d", p=128))
            wk_sb = singles.tile([128, CCH, 2 * D], bf16)
            nc.gpsimd.dma_start(wk_sb[:], wk.ap().rearrange("(cc p) d -> p cc d", p=128))
            wv_sb = singles.tile([128, CCH, 2 * D], bf16)
            nc.gpsimd.dma_start(wv_sb[:], wv.ap().rearrange("(cc p) d -> p cc d", p=128))
            csq_sb = singles.tile([1, 2 * D], bf16)
            nc.gpsimd.dma_start(csq_sb[:], csq.ap())
            csk_sb = singles.tile([1, 2 * D], bf16)
            nc.gpsimd.dma_start(csk_sb[:], csk.ap())
            csv_sb = singles.tile([1, 2 * D], bf16)
            nc.gpsimd.dma_start(csv_sb[:], csv.ap())
            # w_out as [d, h, C] so both heads' lhsT slices sit on partitions 0-63
            wo_sb = singles.tile([D, 2, C], bf16)
            nc.gpsimd.dma_start(wo_sb[:], w_out.ap().rearrange("(h d) c -> d h c", d=D))
            if add_bias:
                qb_sb = singles.tile([128, 1], f32)
                nc.gpsimd.dma_start(qb_sb[:], qb.ap())
                kb_sb = singles.tile([128, 1], f32)
                nc.gpsimd.dma_start(kb_sb[:], kb_b.ap())
                vb_bc = singles.tile([128, 2 * D], f32)
                nc.gpsimd.dma_start(vb_bc[:], vb.ap().to_broadcast((128, 2 * D)))

            # ---------------- x loads (sync queue, chunked) ----------------
            x_view = x.ap().rearrange("(i p) c -> p i c", p=128)
            x_sb = singles.tile([128, NT, C], bf16)
            xT_view = xT.ap().rearrange("(cc p) t -> p cc t", p=128)
            xT_sb = singles.tile([128, CCH, T], bf16)
            for tch in range(TC):
                nc.sync.dma_start(
                    x_sb[:, 4 * tch : 4 * tch + 4, :],
                    x_view[:, 4 * tch : 4 * tch + 4, :],
                )
                nc.sync.dma_start(
                    xT_sb[:, :, 512 * tch : 512 * tch + 512],
                    xT_view[:, :, 512 * tch : 512 * tch + 512],
                )

            # ---------------- persistent tiles ----------------
            mv_sb = singles.tile([128, NT, 2], f32)
            eps_sb = singles.tile([128, 1], f32)
            nc.vector.memset(eps_sb[:], EPS)
            lnv = singles.tile([128, NT], f32)
            rstd = singles.tile([128, NT], f32)  # token-major 1/sqrt(var+eps)
            nmu = singles.tile([128, NT], bf16)  # token-major -mu
            qT_sb = singles.tile([128, T], bf16)
            kT_sb = singles.tile([128, T], bf16)
            v_sb = singles.tile([128, NT, 2, D + 1], bf16)
            nc.gpsimd.memset(v_sb[:], 1.0)  # ones column at [.., D]
            attn_nm = singles.tile([D, 2, T], bf16)
            rstd_bc = singles.tile([128, T], f32)
            nmu_row = singles.tile([1, T], bf16)
            nmu_d = dram.tile([1, T], bf16)
            rstd_d = dram.tile([1, T], f32)
            den_d = dram.tile([8, 512], f32)

            def emit_stats(tch):
                sl = slice(4 * tch, 4 * tch + 4)
                for i in range(4 * tch, 4 * tch + 4):
                    stats = work.tile([128, 6], f32, tag="bnstats")
                    nc.vector.bn_stats(out=stats[:], in_=x_sb[:, i, :])
                    nc.vector.bn_aggr(out=mv_sb[:, i, :], in_=stats[:])
                # rstd = rsqrt(var+eps): bit-hack seed + 2 Newton iters, all
                # on DVE back-to-back (cross-engine hops cost ~1-3us each in
                # queue+semaphore latency, so keep this chain on one engine)
                g = nc.vector
                xv = work.tile([128, 4], f32, tag="nt_x", bufs=2)
                g.tensor_scalar_add(xv[:], mv_sb[:, sl, 1], EPS)
                y = work.tile([128, 4], f32, tag="nt_y", bufs=2)
                yi = y.bitcast(i32)
                g.tensor_scalar(
                    out=yi[:], in0=xv.bitcast(i32)[:], scalar1=1,
                    scalar2=0xFFFFFFFF, op0=OP.logical_shift_right, op1=OP.bitwise_xor,
                )
                g.tensor_scalar_add(yi[:], yi[:], 0x5F3759DF + 1)
                t1 = work.tile([128, 4], f32, tag="nt_t", bufs=2)
                for it in range(2):
                    g.tensor_tensor(out=t1[:], in0=y[:], in1=y[:], op=OP.mult)
                    g.tensor_tensor(out=t1[:], in0=t1[:], in1=xv[:], op=OP.mult)
                    g.tensor_scalar(
                        out=t1[:], in0=t1[:], scalar1=-0.5, scalar2=1.5,
                        op0=OP.mult, op1=OP.add,
                    )
                    dst = rstd[:, sl] if it == 1 else y[:]
                    g.tensor_tensor(out=dst, in0=y[:], in1=t1[:], op=OP.mult)
                g.tensor_scalar_mul(nmu[:, sl], mv_sb[:, sl, 0], -1.0)
                # bounce token-major rows out to DRAM for row/broadcast reloads
                ts = slice(512 * tch, 512 * tch + 512)
                w1 = nc.scalar.dma_start(
                    nmu_d[0:1, ts].rearrange("a (i p) -> p (a i)", p=128), nmu[:, sl]
                )
                w2 = nc.scalar.dma_start(
                    rstd_d[0:1, ts].rearrange("a (i p) -> p (a i)", p=128), rstd[:, sl]
                )
                r1 = nc.scalar.dma_start(nmu_row[:, ts], nmu_d[0:1, ts])
                add_dep_helper(r1.ins, w1.ins, sync=True, reason="nmu bounce")
                r2 = nc.scalar.dma_start(
                    rstd_bc[:, ts], rstd_d[0:1, ts].to_broadcast((128, 512))
                )
                add_dep_helper(r2.ins, w2.ins, sync=True, reason="rstd bounce")

            def emit_qk_proj(tch):
                ts = slice(512 * tch, 512 * tch + 512)
                ps_q = ps_proj.tile([128, 512], f32, tag="proj")
                nc.tensor.matmul(
                    ps_q[:], lhsT=csq_sb[:], rhs=nmu_row[:, ts],
                    start=True, stop=False,
                )
                for cc in range(CCH):
                    nc.tensor.matmul(
                        ps_q[:],
                        lhsT=wq_sb[:, cc, :],
                        rhs=xT_sb[:, cc, ts],
                        start=False,
                        stop=(cc == CCH - 1),
                    )
                nc.vector.tensor_tensor(
                    out=qT_sb[:, ts], in0=ps_q[:], in1=rstd_bc[:, ts], op=OP.mult
                )
                if add_bias:
                    nc.vector.tensor_scalar(
                        out=qT_sb[:, ts], in0=qT_sb[:, ts], scalar1=qb_sb[:],
                        scalar2=None, op0=OP.add,
                    )
                ps_k = ps_proj.tile([128, 512], f32, tag="proj")
                nc.tensor.matmul(
                    ps_k[:], lhsT=csk_sb[:], rhs=nmu_row[:, ts],
                    start=True, stop=False,
                )
                for cc in range(CCH):
                    nc.tensor.matmul(
                        ps_k[:],
                        lhsT=wk_sb[:, cc, :],
                        rhs=xT_sb[:, cc, ts],
                        start=False,
                        stop=(cc == CCH - 1),
                    )
                nc.vector.tensor_tensor(
                    out=kT_sb[:, ts], in0=ps_k[:], in1=rstd_bc[:, ts], op=OP.mult
                )
                if add_bias:
                    nc.gpsimd.tensor_scalar(
                        out=kT_sb[:, ts], in0=kT_sb[:, ts], scalar1=kb_sb[:],
                        scalar2=None, op0=OP.add,
                    )

            def emit_v_proj(tt):
                tsl = slice(128 * tt, 128 * tt + 128)
                ps_v = ps_proj.tile([128, 512], f32, tag="proj")
                nc.tensor.matmul(
                    ps_v[:, 0:128], lhsT=nmu_row[:, tsl], rhs=csv_sb[:],
                    start=True, stop=False,
                )
                for cc in range(CCH):
                    nc.tensor.matmul(
                        ps_v[:, 0:128],
                        lhsT=xT_sb[:, cc, tsl],
                        rhs=wv_sb[:, cc, :],
                        start=False,
                        stop=(cc == CCH - 1),
                    )
                pv = ps_v[:, 0:128].rearrange("p (h d) -> p h d", h=2)
                if tt < 8:
                    # head chunks: ACT is idle before the first softmax exp
                    nc.scalar.mul(v_sb[:, tt, :, 0:D], pv, rstd[:, tt : tt + 1])
                else:
                    nc.vector.tensor_scalar(
                        out=v_sb[:, tt, :, 0:D], in0=pv,
                        scalar1=rstd[:, tt : tt + 1], scalar2=None, op0=OP.mult,
                    )
                if add_bias:
                    nc.gpsimd.tensor_tensor(
                        out=v_sb[:, tt, :, 0:D], in0=v_sb[:, tt, :, 0:D],
                        in1=vb_bc[:].rearrange("p (h d) -> p h d", h=2), op=OP.add,
                    )

            def emit_chunk(tch):
                emit_stats(tch)
                emit_qk_proj(tch)
                for tt in range(4 * tch, 4 * tch + 4):
                    emit_v_proj(tt)

            # ---------------- attention machinery ----------------
            av_tiles = {}

            def emit_event(half, kf, j, h):
                """scores + exp (+ edge memset) + AV for one (key-block, head)."""
                kb = 2 * kf + j
                c0 = 2 * half  # first global chunk of this half
                # visible chunks of this half: c with 2c+1 >= kf
                vis = [c for c in (c0, c0 + 1) if 2 * c + 1 >= kf]
                sc = ps_sc.tile([128, 2, 512], f32, tag="sc")
                e_t = epool.tile([128, 2, 512], bf16, tag="e")
                off = None
                for c in vis:
                    cr = c - c0
                    edge = (kf % 2 == 1) and (c == (kf - 1) // 2)
                    qoff = 256 if edge else 0
                    if edge:
                        off = cr * 512 + 256
                    nc.tensor.matmul(
                        sc[:, cr, qoff:512],
                        lhsT=kT_sb[64 * h : 64 * h + 64, 128 * kb : 128 * kb + 128],
                        rhs=qT_sb[64 * h : 64 * h + 64, 512 * c + qoff : 512 * c + 512],
                        start=True,
                        stop=True,
                    )
                if off is None:
                    off = (vis[0] - c0) * 512
                n = (vis[-1] - c0) * 512 + 512 - off
                sc_flat = sc.rearrange("p a q -> p (a q)")
                e_flat = e_t.rearrange("p a q -> p (a q)")
                nc.scalar.activation(
                    out=e_flat[:, off : off + n], in_=sc_flat[:, off : off + n],
                    func=AF.Exp,
                )
                # zero the masked quarter so AV can run N=512
                for c in vis:
                    cr = c - c0
                    if (kf % 2 == 1) and (c == (kf - 1) // 2):
                        nc.gpsimd.memset(e_t[:, cr, 0:256], 0.0)
                for c in vis:
                    cr = c - c0
                    av = av_tiles[(c, h)]
                    nc.tensor.matmul(
                        av[:],
                        lhsT=v_sb[:, kb, h, :],
                        rhs=e_t[:, cr, :],
                        start=(kb == 0),
                        stop=(kb == 4 * c + 3),
                    )

            def emit_epilogue(c, h):
                """denominator bounce + reciprocal + normalize for (chunk, head)."""
                av = av_tiles[(c, h)]
                hc = h * 4 + c
                den_row = epi.tile([1, 512], f32, tag="den")
                nc.vector.tensor_copy(out=den_row[:], in_=av[D : D + 1, :])
                rec_row = epi.tile([1, 512], f32, tag="drow")
                nc.vector.reciprocal_approx_fast(out=rec_row[:], in_=den_row[:])
                dw = nc.gpsimd.dma_start(den_d[hc : hc + 1, :], rec_row[:])
                rec = epi.tile([D, 512], f32, tag="rec")
                dr = nc.gpsimd.dma_start(
                    rec[:], den_d[hc : hc + 1, :].to_broadcast((D, 512))
                )
                add_dep_helper(dr.ins, dw.ins, sync=True, reason="rec bounce")
                nc.vector.tensor_tensor(
                    out=attn_nm[:, h, 512 * c : 512 * c + 512],
                    in0=av[0:D, :],
                    in1=rec[:],
                    op=OP.mult,
                )

            def emit_outproj(c, Cc):
                ts = slice(512 * c, 512 * c + 512)
                ps_o = ps_proj.tile([128, 512], f32, tag="proj")
                nc.tensor.matmul(
                    ps_o[:],
                    lhsT=wo_sb[:, 0, 128 * Cc : 128 * Cc + 128],
                    rhs=attn_nm[:, 0, ts],
                    start=True,
                    stop=False,
                )
                nc.tensor.matmul(
                    ps_o[:],
                    lhsT=wo_sb[:, 1, 128 * Cc : 128 * Cc + 128],
                    rhs=attn_nm[:, 1, ts],
                    start=False,
                    stop=True,
                )
                stage = work.tile([128, 512], bf16, tag="ostage", bufs=4)
                nc.vector.tensor_copy(out=stage[:], in_=ps_o[:])
                nc.sync.dma_start(out.ap()[128 * Cc : 128 * Cc + 128, ts], stage[:])

            # ---------------- emission schedule ----------------
            emit_chunk(0)
            emit_chunk(1)

            # half 0: key frames 0-3, query chunks 0-1; interleave chunks 2-3
            for c in (0, 1):
                for h in (0, 1):
                    av_tiles[(c, h)] = ps_av.tile([D + 1, 512], f32, tag="av", name=f"av_{c}_{h}")
            fillers = []
            for tch in (2, 3):
                fillers.append(lambda tch=tch: emit_stats(tch))
                fillers.append(lambda tch=tch: emit_qk_proj(tch))
                for tt in range(4 * tch, 4 * tch + 4):
                    fillers.append(lambda tt=tt: emit_v_proj(tt))
            fi = 0
            for kf in range(4):
                for j in range(2):
                    for h in range(2):
                        emit_event(0, kf, j, h)
                    if fi < len(fillers):
                        fillers[fi]()
                        fi += 1
            while fi < len(fillers):
                fillers[fi]()
                fi += 1
            for c in (0, 1):
                for h in (0, 1):
                    emit_epilogue(c, h)

            # half 1: key frames 0-7, query chunks 2-3; interleave outproj c0/c1
            for c in (2, 3):
                for h in (0, 1):
                    av_tiles[(c, h)] = ps_av.tile([D + 1, 512], f32, tag="av", name=f"av_{c}_{h}")
            fillers = [
                (lambda c=c, Cc=Cc: emit_outproj(c, Cc))
                for c in (0, 1)
                for Cc in range(4)
            ]
            fi = 0
            for kf in range(8):
                for j in range(2):
                    for h in range(2):
                        emit_event(1, kf, j, h)
                    if fi < len(fillers):
                        fillers[fi]()
                        fi += 1
            while fi < len(fillers):
                fillers[fi]()
                fi += 1
            for c in (2, 3):
                for h in (0, 1):
                    emit_epilogue(c, h)
            for c in (2, 3):
                for Cc in range(4):
                    emit_outproj(c, Cc)
            if debug_dump:
                dbg_q_sb = singles.tile([128, T], f32)
                nc.vector.tensor_copy(out=dbg_q_sb[:], in_=qT_sb[:])
                nc.sync.dma_start(dbg_q.ap(), dbg_q_sb[:])
                dbg_k_sb = singles.tile([128, T], f32)
                nc.vector.tensor_copy(out=dbg_k_sb[:], in_=kT_sb[:])
                nc.sync.dma_start(dbg_k.ap(), dbg_k_sb[:])
                dbg_v_sb = singles.tile([128, NT * 2 * (D + 1)], f32)
                nc.vector.tensor_copy(
                    out=dbg_v_sb[:], in_=v_sb.rearrange("p a b c -> p (a b c)")
                )
                nc.sync.dma_start(dbg_v.ap(), dbg_v_sb[:])
                dbg_a_sb = singles.tile([D, 2 * T], f32)
                nc.vector.tensor_copy(
                    out=dbg_a_sb[:], in_=attn_nm.rearrange("p a b -> p (a b)")
                )
                nc.sync.dma_start(dbg_a.ap(), dbg_a_sb[:])
                nc.sync.dma_start(dbg_rstd.ap(), rstd[:])
                nc.sync.dma_start(dbg_rec.ap(), den_d[:])

    nc.compile()
    return nc


def _make_in_maps(x, ln_gamma, ln_beta, w_qkv, w_out, b_out):
    bf = ml_dtypes.bfloat16
    x = np.asarray(x, dtype=np.float32)
    g = np.asarray(ln_gamma, dtype=np.float32)
    beta = np.asarray(ln_beta, dtype=np.float32)
    w_qkv = np.asarray(w_qkv, dtype=np.float32)
    w_out = np.asarray(w_out, dtype=np.float32)
    b_out = np.asarray(b_out, dtype=np.float32)

    wg = w_qkv * g[:, None]  # fold gamma into the projection
    bias_qkv = beta @ w_qkv  # fold beta into additive biases
    add_bias = bool(np.any(bias_qkv != 0))
    scale = D ** -0.5

    in_maps = []
    for c in range(N_CORES):
        b, hp = c % 2, c // 2
        qs = slice(128 * hp, 128 * hp + 128)
        ks = slice(INNER + 128 * hp, INNER + 128 * hp + 128)
        vs = slice(2 * INNER + 128 * hp, 2 * INNER + 128 * hp + 128)
        wq_c = wg[:, qs] * scale
        wk_c = wg[:, ks]
        wv_c = wg[:, vs]
        m = {
            "x": np.ascontiguousarray(x[b].astype(bf)),
            "xT": np.ascontiguousarray(x[b].T.astype(bf)),
            "wq": np.ascontiguousarray(wq_c.astype(bf)),
            "wk": np.ascontiguousarray(wk_c.astype(bf)),
            "wv": np.ascontiguousarray(wv_c.astype(bf)),
            "csq": np.ascontiguousarray(wq_c.sum(axis=0).reshape(1, 128).astype(bf)),
            "csk": np.ascontiguousarray(wk_c.sum(axis=0).reshape(1, 128).astype(bf)),
            "csv": np.ascontiguousarray(wv_c.sum(axis=0).reshape(1, 128).astype(bf)),
            "w_out": np.ascontiguousarray(w_out[128 * hp : 128 * (hp + 1), :].astype(bf)),
        }
        if add_bias:
            m["qb"] = np.ascontiguousarray(
                (bias_qkv[qs] * scale).reshape(128, 1).astype(np.float32)
            )
            m["kb"] = np.ascontiguousarray(bias_qkv[ks].reshape(128, 1).astype(np.float32))
            m["vb"] = np.ascontiguousarray(bias_qkv[vs].reshape(1, 128).astype(np.float32))
        in_maps.append(m)
    return in_maps, add_bias, b_out


def _run(inputs, trace=False, trace_cores=None):
    from concourse.bass_utils import run_bass_kernel_spmd

    in_maps, add_bias, b_out = _make_in_maps(**inputs)
    key = ("nc", add_bias)
    if key not in _CACHE:
        _CACHE[key] = _build(add_bias)
    nc = _CACHE[key]
    res = run_bass_kernel_spmd(
        nc,
        in_maps,
        core_ids=list(range(N_CORES)),
        trace=trace,
        trace_cores=trace_cores,
    )
    # sum-unshard: out-projection is row-parallel across head-pairs; each core
    # returns an outT [C, T] bf16 partial; host transposes, sums, adds bias.
    full = np.zeros((B, T, C), dtype=np.float32)
    for c in range(N_CORES):
        full[c % 2] += res.results[c]["out"].astype(np.float32).T
    full += b_out.reshape(1, 1, C)
    return full, res


def kernel(**inputs):
    full, _ = _run(inputs, trace=False)
    return full


# revision 25
# speedup vs baseline: 1.4140x; 1.4140x over previous
"""Distributed Trainium2 Bass kernel for block-causal multi-head attention.

Problem: LayerNorm -> QKV projection -> 8-head attention with block-causal mask
(8 frames x 256 patches) -> output projection + bias.  x: [2, 2048, 512] f32.

Sharding (8 cores): core c handles batch b = c%2 and head-pair hp = c//2
(heads 2hp, 2hp+1).

Key structure (v2 - no PE transposes, fat matmuls, ACT-exp-bound):
  - host ships x BOTH token-major (stats) and transposed xT (projections),
    both bf16; gamma and 1/sqrt(d) folded into weights; weight column-sums
    shipped for the LN mean correction
  - LN: bn_stats on token-major x; rstd = Exp(-0.5*Ln(var+eps)) (one ACT
    table set shared with softmax exp); mean correction enters each
    projection as a rank-1 matmul seed (-mu ganger colsum_w) accumulated into
    the same PSUM as the raw projection
  - Q/K projected feature-major (rhs = xT, N=512 matmuls); V token-major
    (lhsT = xT chunks); rstd applied via one broadcast multiply (Q),
    per-partition tensor_scalar (V), and the softmax exp's per-partition
    scale operand (K - free)
  - scores per (key-block, head) at N=512 into a 2-bank PSUM tile; ONE exp
    per (kb, h) covering only visible elements; AV accumulates per 512-query
    chunk with a ones-column denominator row (M=65); block-causal blocks
    skipped at chunk granularity, odd-frame edges via N=256 matmuls + e=0
    memsets
  - denominator: DRAM-bounce broadcast, fast reciprocal, normalize into
    attn_nm bf16; out-projection computed TRANSPOSED (out^T = w_out^T @
    attn^T, lhsT = w_out slices) -> outT [C, T] bf16 partials; host
    transposes, sums the 4 head-pair partials per batch and adds b_out
"""

import numpy as np
import ml_dtypes

B = 2
T = 2048
C = 512
H = 8
D = 64
INNER = H * D  # 512
NP = 256  # patches per frame
F = 8  # frames
EPS = 1e-5
N_CORES = 8
NT = T // 128  # 16 token tiles of 128
TC = T // 512  # 4 token chunks of 512
CCH = C // 128  # 4 channel chunks of 128

_CACHE = {}


def _build(add_bias: bool, debug_dump: bool = False):
    import concourse.bass as bass
    import concourse.tile as tile
    from concourse import bacc, mybir

    f32 = mybir.dt.float32
    bf16 = mybir.dt.bfloat16
    i32 = mybir.dt.int32
    AF = mybir.ActivationFunctionType
    OP = mybir.AluOpType

    from concourse.tile_rust import add_dep_helper

    nc = bacc.Bacc("TRN2", target_bir_lowering=False, debug=False, num_devices=N_CORES)

    x = nc.dram_tensor("x", [T, C], bf16, kind="ExternalInput")
    xT = nc.dram_tensor("xT", [C, T], bf16, kind="ExternalInput")
    wq = nc.dram_tensor("wq", [C, 2 * D], bf16, kind="ExternalInput")
    wk = nc.dram_tensor("wk", [C, 2 * D], bf16, kind="ExternalInput")
    wv = nc.dram_tensor("wv", [C, 2 * D], bf16, kind="ExternalInput")
    csq = nc.dram_tensor("csq", [4, 4 * 2 * D], bf16, kind="ExternalInput")
    csk = nc.dram_tensor("csk", [4, 4 * 2 * D], bf16, kind="ExternalInput")
    csv = nc.dram_tensor("csv", [4, 4 * 2 * D], bf16, kind="ExternalInput")
    sel4 = nc.dram_tensor("sel4", [4, 4 * 2 * D], f32, kind="ExternalInput")
    w_out = nc.dram_tensor("w_out", [2 * D, C], bf16, kind="ExternalInput")
    if add_bias:
        qb = nc.dram_tensor("qb", [2 * D, 1], f32, kind="ExternalInput")
        kb_b = nc.dram_tensor("kb", [2 * D, 1], f32, kind="ExternalInput")
        vb = nc.dram_tensor("vb", [1, 2 * D], f32, kind="ExternalInput")
    out = nc.dram_tensor("out", [C, T], bf16, kind="ExternalOutput")
    if debug_dump:
        dbg_rstd = nc.dram_tensor("dbg_rstd", [128, NT], f32, kind="ExternalOutput")
        dbg_q = nc.dram_tensor("dbg_q", [128, T], f32, kind="ExternalOutput")
        dbg_k = nc.dram_tensor("dbg_k", [128, T], f32, kind="ExternalOutput")
        dbg_v = nc.dram_tensor("dbg_v", [128, NT * 2 * 128], f32, kind="ExternalOutput")
        dbg_a = nc.dram_tensor("dbg_a", [D, 2 * T], f32, kind="ExternalOutput")

    with tile.TileContext(nc) as tc:
        import contextlib

        with contextlib.ExitStack() as ctx:
            singles = ctx.enter_context(tc.tile_pool(name="singles", bufs=1))
            work = ctx.enter_context(tc.tile_pool(name="work", bufs=2))
            epool = ctx.enter_context(tc.tile_pool(name="epool", bufs=3))
            epi = ctx.enter_context(tc.tile_pool(name="epi", bufs=2))
            ps_proj = ctx.enter_context(tc.tile_pool(name="ps_proj", bufs=2, space="PSUM"))
            ps_sc = ctx.enter_context(tc.tile_pool(name="ps_sc", bufs=1, space="PSUM"))
            ps_av = ctx.enter_context(tc.tile_pool(name="ps_av", bufs=4, space="PSUM"))
            dram = ctx.enter_context(tc.tile_pool(name="dram", bufs=1, space="DRAM"))

            # ---------------- weights (gpsimd queue) ----------------
            wq_sb = singles.tile([128, CCH, 2 * D], bf16)
            nc.gpsimd.dma_start(wq_sb[:], wq.ap().rearrange("(cc p) d -> p cc d", p=128))
            wk_sb = singles.tile([128, CCH, 2 * D], bf16)
            nc.gpsimd.dma_start(wk_sb[:], wk.ap().rearrange("(cc p) d -> p cc d", p=128))
            wv_sb = singles.tile([128, CCH, 2 * D], bf16)
            nc.gpsimd.dma_start(wv_sb[:], wv.ap().rearrange("(cc p) d -> p cc d", p=128))
            csq_sb = singles.tile([4, 4, 2 * D], bf16)
            nc.gpsimd.dma_start(csq_sb[:], csq.ap().rearrange("k (li d) -> k li d", li=4))
            csk_sb = singles.tile([4, 4, 2 * D], bf16)
            nc.gpsimd.dma_start(csk_sb[:], csk.ap().rearrange("k (li d) -> k li d", li=4))
            csv_sb = singles.tile([4, 4, 2 * D], bf16)
            nc.gpsimd.dma_start(csv_sb[:], csv.ap().rearrange("k (li d) -> k li d", li=4))
            sel4_sb = singles.tile([4, 4, 2 * D], f32)
            nc.gpsimd.dma_start(sel4_sb[:], sel4.ap().rearrange("k (li d) -> k li d", li=4))
            # w_out as [d, h, C] so both heads' lhsT slices sit on partitions 0-63
            wo_sb = singles.tile([D, 2, C], bf16)
            nc.gpsimd.dma_start(wo_sb[:], w_out.ap().rearrange("(h d) c -> d h c", d=D))
            if add_bias:
                qb_sb = singles.tile([128, 1], f32)
                nc.gpsimd.dma_start(qb_sb[:], qb.ap())
                kb_sb = singles.tile([128, 1], f32)
                nc.gpsimd.dma_start(kb_sb[:], kb_b.ap())
                vb_bc = singles.tile([128, 2 * D], f32)
                nc.gpsimd.dma_start(vb_bc[:], vb.ap().to_broadcast((128, 2 * D)))

            # ---------------- x loads (sync queue, chunked) ----------------
            x_view = x.ap().rearrange("(i p) c -> p i c", p=128)
            x_sb = singles.tile([128, NT, C], bf16)
            xT_view = xT.ap().rearrange("(cc p) t -> p cc t", p=128)
            xT_sb = singles.tile([128, CCH, T], bf16)
            for tch in range(TC):
                nc.sync.dma_start(
                    x_sb[:, 4 * tch : 4 * tch + 4, :],
                    x_view[:, 4 * tch : 4 * tch + 4, :],
                )
                nc.sync.dma_start(
                    xT_sb[:, :, 512 * tch : 512 * tch + 512],
                    xT_view[:, :, 512 * tch : 512 * tch + 512],
                )

            # ---------------- persistent tiles ----------------
            mv_sb = singles.tile([128, NT, 2], f32)
            eps_sb = singles.tile([128, 1], f32)
            nc.vector.memset(eps_sb[:], EPS)
            lnv = singles.tile([128, NT], f32)
            rstd = singles.tile([128, NT], f32)  # token-major 1/sqrt(var+eps)
            nmu = singles.tile([128, NT], f32)  # token-major -mu
            qT_sb = singles.tile([128, T], bf16)
            kT_sb = singles.tile([128, T], bf16)
            v_sb = singles.tile([128, NT, 2, 128], bf16)
            nc.gpsimd.memset(v_sb[:], 1.0)  # ones column at [.., D]
            attn_nm = singles.tile([D, 2, T], bf16)
            rstd_bc = singles.tile([128, T], f32)
            nmuT_sb = singles.tile([4, TC, 128], bf16)
            rstdT_sb = singles.tile([4, TC, 128], f32)
            from concourse.masks import make_identity
            ident_f32 = singles.tile([128, 128], f32)
            make_identity(nc, ident_f32[:])

            def emit_stats(tch):
                sl = slice(4 * tch, 4 * tch + 4)
                for i in range(4 * tch, 4 * tch + 4):
                    stats = work.tile([128, 6], f32, tag="bnstats")
                    nc.vector.bn_stats(out=stats[:], in_=x_sb[:, i, :])
                    nc.vector.bn_aggr(out=mv_sb[:, i, :], in_=stats[:])
                # rstd = rsqrt(var+eps): bit-hack seed + 2 Newton iters, all
                # on DVE back-to-back (cross-engine hops cost ~1-3us each in
                # queue+semaphore latency, so keep this chain on one engine)
                g = nc.vector
                xv = work.tile([128, 4], f32, tag="nt_x", bufs=2)
                g.tensor_scalar_add(xv[:], mv_sb[:, sl, 1], EPS)
                y = work.tile([128, 4], f32, tag="nt_y", bufs=2)
                yi = y.bitcast(i32)
                g.tensor_scalar(
                    out=yi[:], in0=xv.bitcast(i32)[:], scalar1=1,
                    scalar2=0xFFFFFFFF, op0=OP.logical_shift_right, op1=OP.bitwise_xor,
                )
                g.tensor_scalar_add(yi[:], yi[:], 0x5F3759DF + 1)
                t1 = work.tile([128, 4], f32, tag="nt_t", bufs=2)
                for it in range(2):
                    g.tensor_tensor(out=t1[:], in0=y[:], in1=y[:], op=OP.mult)
                    g.tensor_tensor(out=t1[:], in0=t1[:], in1=xv[:], op=OP.mult)
                    g.tensor_scalar(
                        out=t1[:], in0=t1[:], scalar1=-0.5, scalar2=1.5,
                        op0=OP.mult, op1=OP.add,
                    )
                    dst = rstd[:, sl] if it == 1 else y[:]
                    g.tensor_tensor(out=dst, in0=y[:], in1=t1[:], op=OP.mult)
                g.tensor_scalar_mul(nmu[:, sl], mv_sb[:, sl, 0], -1.0)
                # stat rows on-chip: transpose [128,4] token-major stats into
                # [4,128] row pieces (one partition per 128-token tile), then
                # build the rstd column-broadcast with K=1 outer matmuls
                ps_t = ps_proj.tile([128, 512], f32, tag="proj", name=f"ps_t{tch}")
                nc.tensor.transpose(ps_t[0:4, 0:128], nmu[:, sl], ident_f32[:])
                nc.tensor.transpose(ps_t[0:4, 128:256], rstd[:, sl], ident_f32[:])
                nc.vector.tensor_copy(out=nmuT_sb[:, tch, :], in_=ps_t[0:4, 0:128])
                nc.vector.tensor_copy(out=rstdT_sb[:, tch, :], in_=ps_t[0:4, 128:256])
                ps_bc = ps_proj.tile([128, 512], f32, tag="proj", name=f"ps_bc{tch}")
                for li in range(4):
                    nc.tensor.matmul(
                        ps_bc[:, 128 * li : 128 * li + 128],
                        lhsT=sel4_sb[:, li, :],
                        rhs=rstdT_sb[:, tch, :],
                        start=True,
                        stop=True,
                    )
                ts = slice(512 * tch, 512 * tch + 512)
                nc.vector.tensor_copy(out=rstd_bc[:, ts], in_=ps_bc[:])

            def emit_qk_proj(tch):
                ts = slice(512 * tch, 512 * tch + 512)
                ps_q = ps_proj.tile([128, 512], f32, tag="proj")
                for cc in range(CCH):
                    nc.tensor.matmul(
                        ps_q[:],
                        lhsT=wq_sb[:, cc, :],
                        rhs=xT_sb[:, cc, ts],
                        start=(cc == 0),
                        stop=False,
                    )
                for li in range(4):
                    nc.tensor.matmul(
                        ps_q[:, 128 * li : 128 * li + 128],
                        lhsT=csq_sb[:, li, :],
                        rhs=nmuT_sb[:, tch, :],
                        start=False,
                        stop=(li == 3),
                    )
                nc.vector.tensor_tensor(
                    out=qT_sb[:, ts], in0=ps_q[:], in1=rstd_bc[:, ts], op=OP.mult
                )
                if add_bias:
                    nc.vector.tensor_scalar(
                        out=qT_sb[:, ts], in0=qT_sb[:, ts], scalar1=qb_sb[:],
                        scalar2=None, op0=OP.add,
                    )
                ps_k = ps_proj.tile([128, 512], f32, tag="proj")
                for cc in range(CCH):
                    nc.tensor.matmul(
                        ps_k[:],
                        lhsT=wk_sb[:, cc, :],
                        rhs=xT_sb[:, cc, ts],
                        start=(cc == 0),
                        stop=False,
                    )
                for li in range(4):
                    nc.tensor.matmul(
                        ps_k[:, 128 * li : 128 * li + 128],
                        lhsT=csk_sb[:, li, :],
                        rhs=nmuT_sb[:, tch, :],
                        start=False,
                        stop=(li == 3),
                    )
                nc.vector.tensor_tensor(
                    out=kT_sb[:, ts], in0=ps_k[:], in1=rstd_bc[:, ts], op=OP.mult
                )
                if add_bias:
                    nc.gpsimd.tensor_scalar(
                        out=kT_sb[:, ts], in0=kT_sb[:, ts], scalar1=kb_sb[:],
                        scalar2=None, op0=OP.add,
                    )

            def emit_v_proj(tt):
                tsl = slice(128 * tt, 128 * tt + 128)
                ps_v = ps_proj.tile([128, 512], f32, tag="proj")
                for cc in range(CCH):
                    nc.tensor.matmul(
                        ps_v[:, 0:128],
                        lhsT=xT_sb[:, cc, tsl],
                        rhs=wv_sb[:, cc, :],
                        start=(cc == 0),
                        stop=False,
                    )
                nc.tensor.matmul(
                    ps_v[:, 0:128],
                    lhsT=nmuT_sb[:, tt // 4, :],
                    rhs=csv_sb[:, tt % 4, :],
                    start=False,
                    stop=True,
                )
                pv = ps_v[:, 0:128].rearrange("p (h d) -> p h d", h=2)
                if tt < 8:
                    # head chunks: ACT is idle before the first softmax exp
                    nc.scalar.mul(v_sb[:, tt, :, 0:D], pv, rstd[:, tt : tt + 1])
                else:
                    nc.vector.tensor_scalar(
                        out=v_sb[:, tt, :, 0:D], in0=pv,
                        scalar1=rstd[:, tt : tt + 1], scalar2=None, op0=OP.mult,
                    )
                if add_bias:
                    nc.gpsimd.tensor_tensor(
                        out=v_sb[:, tt, :, 0:D], in0=v_sb[:, tt, :, 0:D],
                        in1=vb_bc[:].rearrange("p (h d) -> p h d", h=2), op=OP.add,
                    )

            def emit_chunk(tch):
                emit_stats(tch)
                emit_qk_proj(tch)
                for tt in range(4 * tch, 4 * tch + 4):
                    emit_v_proj(tt)

            # ---------------- attention machinery ----------------
            av_tiles = {}

            def emit_event(half, kf, j, h):
                """scores + exp (+ edge memset) + AV for one (key-block, head)."""
                kb = 2 * kf + j
                c0 = 2 * half  # first global chunk of this half
                # visible chunks of this half: c with 2c+1 >= kf
                vis = [c for c in (c0, c0 + 1) if 2 * c + 1 >= kf]
                sc = ps_sc.tile([128, 2, 512], f32, tag="sc")
                e_t = epool.tile([128, 2, 512], bf16, tag="e")
                off = None
                for c in vis:
                    cr = c - c0
                    edge = (kf % 2 == 1) and (c == (kf - 1) // 2)
                    qoff = 256 if edge else 0
                    if edge:
                        off = cr * 512 + 256
                    nc.tensor.matmul(
                        sc[:, cr, qoff:512],
                        lhsT=kT_sb[64 * h : 64 * h + 64, 128 * kb : 128 * kb + 128],
                        rhs=qT_sb[64 * h : 64 * h + 64, 512 * c + qoff : 512 * c + 512],
                        start=True,
                        stop=True,
                    )
                if off is None:
                    off = (vis[0] - c0) * 512
                n = (vis[-1] - c0) * 512 + 512 - off
                sc_flat = sc.rearrange("p a q -> p (a q)")
                e_flat = e_t.rearrange("p a q -> p (a q)")
                nc.scalar.activation(
                    out=e_flat[:, off : off + n], in_=sc_flat[:, off : off + n],
                    func=AF.Exp,
                )
                # zero the masked quarter so AV can run N=512
                for c in vis:
                    cr = c - c0
                    if (kf % 2 == 1) and (c == (kf - 1) // 2):
                        nc.gpsimd.memset(e_t[:, cr, 0:256], 0.0)
                for c in vis:
                    cr = c - c0
                    av = av_tiles[(c, h)]
                    nc.tensor.matmul(
                        av[:],
                        lhsT=v_sb[:, kb, h, :],
                        rhs=e_t[:, cr, :],
                        start=(kb == 0),
                        stop=(kb == 4 * c + 3),
                    )

            def emit_epilogue(c, h):
                """denominator bounce + reciprocal + normalize for (chunk, head)."""
                av = av_tiles[(c, h)]
                den = epi.tile([D, 512], f32, tag="den")
                nc.vector.tensor_copy(out=den[:], in_=av[D : 2 * D, :])
                rec = epi.tile([D, 512], f32, tag="rec")
                nc.vector.reciprocal_approx_fast(out=rec[:], in_=den[:])
                nc.vector.tensor_tensor(
                    out=attn_nm[:, h, 512 * c : 512 * c + 512],
                    in0=av[0:D, :],
                    in1=rec[:],
                    op=OP.mult,
                )

            def emit_outproj(c, Cc):
                ts = slice(512 * c, 512 * c + 512)
                ps_o = ps_proj.tile([128, 512], f32, tag="proj")
                nc.tensor.matmul(
                    ps_o[:],
                    lhsT=wo_sb[:, 0, 128 * Cc : 128 * Cc + 128],
                    rhs=attn_nm[:, 0, ts],
                    start=True,
                    stop=False,
                )
                nc.tensor.matmul(
                    ps_o[:],
                    lhsT=wo_sb[:, 1, 128 * Cc : 128 * Cc + 128],
                    rhs=attn_nm[:, 1, ts],
                    start=False,
                    stop=True,
                )
                stage = work.tile([128, 512], bf16, tag="ostage", bufs=4)
                nc.vector.tensor_copy(out=stage[:], in_=ps_o[:])
                nc.sync.dma_start(out.ap()[128 * Cc : 128 * Cc + 128, ts], stage[:])

            # ---------------- emission schedule ----------------
            emit_chunk(0)
            emit_chunk(1)

            # half 0: key frames 0-3, query chunks 0-1; interleave chunks 2-3
            for c in (0, 1):
                for h in (0, 1):
                    av_tiles[(c, h)] = ps_av.tile([128, 512], f32, tag="av", name=f"av_{c}_{h}")
            fillers = []
            for tch in (2, 3):
                fillers.append(lambda tch=tch: emit_stats(tch))
                fillers.append(lambda tch=tch: emit_qk_proj(tch))
                for tt in range(4 * tch, 4 * tch + 4):
                    fillers.append(lambda tt=tt: emit_v_proj(tt))
            fi = 0
            for kf in range(4):
                for j in range(2):
                    for h in range(2):
                        emit_event(0, kf, j, h)
                    if fi < len(fillers):
                        fillers[fi]()
                        fi += 1
            while fi < len(fillers):
                fillers[fi]()
                fi += 1
            for c in (0, 1):
                for h in (0, 1):
                    emit_epilogue(c, h)

            # half 1: key frames 0-7, query chunks 2-3; interleave outproj c0/c1
            for c in (2, 3):
                for h in (0, 1):
                    av_tiles[(c, h)] = ps_av.tile([128, 512], f32, tag="av", name=f"av_{c}_{h}")
            fillers = [
                (lambda c=c, Cc=Cc: emit_outproj(c, Cc))
                for c in (0, 1)
                for Cc in range(4)
            ]
            fi = 0
            for kf in range(8):
                for j in range(2):
                    for h in range(2):
                        emit_event(1, kf, j, h)
                    if fi < len(fillers):
                        fillers[fi]()
                        fi += 1
            while fi < len(fillers):
                fillers[fi]()
                fi += 1
            for c in (2, 3):
                for h in (0, 1):
                    emit_epilogue(c, h)
            for c in (2, 3):
                for Cc in range(4):
                    emit_outproj(c, Cc)
            if debug_dump:
                dbg_q_sb = singles.tile([128, T], f32)
                nc.vector.tensor_copy(out=dbg_q_sb[:], in_=qT_sb[:])
                nc.sync.dma_start(dbg_q.ap(), dbg_q_sb[:])
                dbg_k_sb = singles.tile([128, T], f32)
                nc.vector.tensor_copy(out=dbg_k_sb[:], in_=kT_sb[:])
                nc.sync.dma_start(dbg_k.ap(), dbg_k_sb[:])
                dbg_v_sb = singles.tile([128, NT * 2 * 128], f32)
                nc.vector.tensor_copy(
                    out=dbg_v_sb[:], in_=v_sb.rearrange("p a b c -> p (a b c)")
                )
                nc.sync.dma_start(dbg_v.ap(), dbg_v_sb[:])
                dbg_a_sb = singles.tile([D, 2 * T], f32)
                nc.vector.tensor_copy(
                    out=dbg_a_sb[:], in_=attn_nm.rearrange("p a b -> p (a b)")
                )
                nc.sync.dma_start(dbg_a.ap(), dbg_a_sb[:])
                nc.sync.dma_start(dbg_rstd.ap(), rstd[:])

    nc.compile()
    return nc


def _sel_rows(v):
    # [4, 4*128]: block li holds v on row k==li, zeros elsewhere
    out = np.zeros((4, 4 * 128), dtype=np.float32)
    for li in range(4):
        out[li, 128 * li : 128 * li + 128] = v
    return np.ascontiguousarray(out)


def _make_in_maps(x, ln_gamma, ln_beta, w_qkv, w_out, b_out):
    bf = ml_dtypes.bfloat16
    x = np.asarray(x, dtype=np.float32)
    g = np.asarray(ln_gamma, dtype=np.float32)
    beta = np.asarray(ln_beta, dtype=np.float32)
    w_qkv = np.asarray(w_qkv, dtype=np.float32)
    w_out = np.asarray(w_out, dtype=np.float32)
    b_out = np.asarray(b_out, dtype=np.float32)

    wg = w_qkv * g[:, None]  # fold gamma into the projection
    bias_qkv = beta @ w_qkv  # fold beta into additive biases
    add_bias = bool(np.any(bias_qkv != 0))
    scale = D ** -0.5

    in_maps = []
    for c in range(N_CORES):
        b, hp = c % 2, c // 2
        qs = slice(128 * hp, 128 * hp + 128)
        ks = slice(INNER + 128 * hp, INNER + 128 * hp + 128)
        vs = slice(2 * INNER + 128 * hp, 2 * INNER + 128 * hp + 128)
        wq_c = wg[:, qs] * scale
        wk_c = wg[:, ks]
        wv_c = wg[:, vs]
        m = {
            "x": np.ascontiguousarray(x[b].astype(bf)),
            "xT": np.ascontiguousarray(x[b].T.astype(bf)),
            "wq": np.ascontiguousarray(wq_c.astype(bf)),
            "wk": np.ascontiguousarray(wk_c.astype(bf)),
            "wv": np.ascontiguousarray(wv_c.astype(bf)),
            "csq": _sel_rows(wq_c.sum(axis=0)).astype(bf),
            "csk": _sel_rows(wk_c.sum(axis=0)).astype(bf),
            "csv": _sel_rows(wv_c.sum(axis=0)).astype(bf),
            "sel4": _sel_rows(np.ones(128, dtype=np.float32)).astype(np.float32),
            "w_out": np.ascontiguousarray(w_out[128 * hp : 128 * (hp + 1), :].astype(bf)),
        }
        if add_bias:
            m["qb"] = np.ascontiguousarray(
                (bias_qkv[qs] * scale).reshape(128, 1).astype(np.float32)
            )
            m["kb"] = np.ascontiguousarray(bias_qkv[ks].reshape(128, 1).astype(np.float32))
            m["vb"] = np.ascontiguousarray(bias_qkv[vs].reshape(1, 128).astype(np.float32))
        in_maps.append(m)
    return in_maps, add_bias, b_out


def _run(inputs, trace=False, trace_cores=None):
    from concourse.bass_utils import run_bass_kernel_spmd

    in_maps, add_bias, b_out = _make_in_maps(**inputs)
    key = ("nc", add_bias)
    if key not in _CACHE:
        _CACHE[key] = _build(add_bias)
    nc = _CACHE[key]
    res = run_bass_kernel_spmd(
        nc,
        in_maps,
        core_ids=list(range(N_CORES)),
        trace=trace,
        trace_cores=trace_cores,
    )
    # sum-unshard: out-projection is row-parallel across head-pairs; each core
    # returns an outT [C, T] bf16 partial; host transposes, sums, adds bias.
    full = np.zeros((B, T, C), dtype=np.float32)
    for c in range(N_CORES):
        full[c % 2] += res.results[c]["out"].astype(np.float32).T
    full += b_out.reshape(1, 1, C)
    return full, res


def kernel(**inputs):
    full, _ = _run(inputs, trace=False)
    return full


# revision 26
# speedup vs baseline: 2.1388x; 1.5126x over previous
"""Distributed Trainium2 Bass kernel for block-causal multi-head attention.

Problem: LayerNorm -> QKV projection -> 8-head attention with block-causal mask
(8 frames x 256 patches) -> output projection + bias.  x: [2, 2048, 512] f32.

Sharding (8 cores): core c handles batch b = c%2 and head-pair hp = c//2
(heads 2hp, 2hp+1).

Key structure (v2 - no PE transposes, fat matmuls, ACT-exp-bound):
  - host ships x BOTH token-major (stats) and transposed xT (projections),
    both bf16; gamma and 1/sqrt(d) folded into weights; weight column-sums
    shipped for the LN mean correction
  - LN: bn_stats on token-major x; rstd = Exp(-0.5*Ln(var+eps)) (one ACT
    table set shared with softmax exp); mean correction enters each
    projection as a rank-1 matmul seed (-mu ganger colsum_w) accumulated into
    the same PSUM as the raw projection
  - Q/K projected feature-major (rhs = xT, N=512 matmuls); V token-major
    (lhsT = xT chunks); rstd applied via one broadcast multiply (Q),
    per-partition tensor_scalar (V), and the softmax exp's per-partition
    scale operand (K - free)
  - scores per (key-block, head) at N=512 into a 2-bank PSUM tile; ONE exp
    per (kb, h) covering only visible elements; AV accumulates per 512-query
    chunk with a ones-column denominator row (M=65); block-causal blocks
    skipped at chunk granularity, odd-frame edges via N=256 matmuls + e=0
    memsets
  - denominator: DRAM-bounce broadcast, fast reciprocal, normalize into
    attn_nm bf16; out-projection computed TRANSPOSED (out^T = w_out^T @
    attn^T, lhsT = w_out slices) -> outT [C, T] bf16 partials; host
    transposes, sums the 4 head-pair partials per batch and adds b_out
"""

import numpy as np
import ml_dtypes

B = 2
T = 2048
C = 512
H = 8
D = 64
INNER = H * D  # 512
NP = 256  # patches per frame
F = 8  # frames
EPS = 1e-5
N_CORES = 8
NT = T // 128  # 16 token tiles of 128
TC = T // 512  # 4 token chunks of 512
CCH = C // 128  # 4 channel chunks of 128

_CACHE = {}


def _build(add_bias: bool, debug_dump: bool = False):
    import concourse.bass as bass
    import concourse.tile as tile
    from concourse import bacc, mybir

    f32 = mybir.dt.float32
    bf16 = mybir.dt.bfloat16
    i32 = mybir.dt.int32
    AF = mybir.ActivationFunctionType
    OP = mybir.AluOpType

    from concourse.tile_rust import add_dep_helper

    nc = bacc.Bacc("TRN2", target_bir_lowering=False, debug=False, num_devices=N_CORES)

    x = nc.dram_tensor("x", [T, C], bf16, kind="ExternalInput")
    xT = nc.dram_tensor("xT", [C, T], bf16, kind="ExternalInput")
    wq = nc.dram_tensor("wq", [C, 2 * D], bf16, kind="ExternalInput")
    wk = nc.dram_tensor("wk", [C, 2 * D], bf16, kind="ExternalInput")
    wv = nc.dram_tensor("wv", [C, 2 * D], bf16, kind="ExternalInput")
    csq = nc.dram_tensor("csq", [4, 4 * 2 * D], bf16, kind="ExternalInput")
    csk = nc.dram_tensor("csk", [4, 4 * 2 * D], bf16, kind="ExternalInput")
    csv = nc.dram_tensor("csv", [4, 4 * 2 * D], bf16, kind="ExternalInput")
    sel4 = nc.dram_tensor("sel4", [4, 4 * 2 * D], f32, kind="ExternalInput")
    w_out = nc.dram_tensor("w_out", [2 * D, C], bf16, kind="ExternalInput")
    if add_bias:
        qb = nc.dram_tensor("qb", [2 * D, 1], f32, kind="ExternalInput")
        kb_b = nc.dram_tensor("kb", [2 * D, 1], f32, kind="ExternalInput")
        vb = nc.dram_tensor("vb", [1, 2 * D], f32, kind="ExternalInput")
    out = nc.dram_tensor("out", [C, T], bf16, kind="ExternalOutput")
    if debug_dump:
        dbg_rstd = nc.dram_tensor("dbg_rstd", [128, NT], f32, kind="ExternalOutput")
        dbg_q = nc.dram_tensor("dbg_q", [128, T], f32, kind="ExternalOutput")
        dbg_k = nc.dram_tensor("dbg_k", [128, T], f32, kind="ExternalOutput")
        dbg_v = nc.dram_tensor("dbg_v", [128, NT * 2 * 128], f32, kind="ExternalOutput")
        dbg_a = nc.dram_tensor("dbg_a", [D, 2 * T], f32, kind="ExternalOutput")

    with tile.TileContext(nc) as tc:
        import contextlib

        with contextlib.ExitStack() as ctx:
            singles = ctx.enter_context(tc.tile_pool(name="singles", bufs=1))
            work = ctx.enter_context(tc.tile_pool(name="work", bufs=2))
            epool = ctx.enter_context(tc.tile_pool(name="epool", bufs=3))
            epi = ctx.enter_context(tc.tile_pool(name="epi", bufs=2))
            ps_proj = ctx.enter_context(tc.tile_pool(name="ps_proj", bufs=2, space="PSUM"))
            ps_sc = ctx.enter_context(tc.tile_pool(name="ps_sc", bufs=2, space="PSUM"))
            ps_av = ctx.enter_context(tc.tile_pool(name="ps_av", bufs=1, space="PSUM"))
            dram = ctx.enter_context(tc.tile_pool(name="dram", bufs=1, space="DRAM"))

            # ---------------- weights (gpsimd queue) ----------------
            wq_sb = singles.tile([128, CCH, 2 * D], bf16)
            nc.gpsimd.dma_start(wq_sb[:], wq.ap().rearrange("(cc p) d -> p cc d", p=128))
            wk_sb = singles.tile([128, CCH, 2 * D], bf16)
            nc.gpsimd.dma_start(wk_sb[:], wk.ap().rearrange("(cc p) d -> p cc d", p=128))
            wv_sb = singles.tile([128, CCH, 2 * D], bf16)
            nc.gpsimd.dma_start(wv_sb[:], wv.ap().rearrange("(cc p) d -> p cc d", p=128))
            csq_sb = singles.tile([4, 4, 2 * D], bf16)
            nc.gpsimd.dma_start(csq_sb[:], csq.ap().rearrange("k (li d) -> k li d", li=4))
            csk_sb = singles.tile([4, 4, 2 * D], bf16)
            nc.gpsimd.dma_start(csk_sb[:], csk.ap().rearrange("k (li d) -> k li d", li=4))
            csv_sb = singles.tile([4, 4, 2 * D], bf16)
            nc.gpsimd.dma_start(csv_sb[:], csv.ap().rearrange("k (li d) -> k li d", li=4))
            sel4_sb = singles.tile([4, 4, 2 * D], f32)
            nc.gpsimd.dma_start(sel4_sb[:], sel4.ap().rearrange("k (li d) -> k li d", li=4))
            # w_out as [d, h, C] so both heads' lhsT slices sit on partitions 0-63
            wo_sb = singles.tile([D, 2, C], bf16)
            nc.gpsimd.dma_start(wo_sb[:], w_out.ap().rearrange("(h d) c -> d h c", d=D))
            if add_bias:
                qb_sb = singles.tile([128, 1], f32)
                nc.gpsimd.dma_start(qb_sb[:], qb.ap())
                kb_sb = singles.tile([128, 1], f32)
                nc.gpsimd.dma_start(kb_sb[:], kb_b.ap())
                vb_bc = singles.tile([128, 2 * D], f32)
                nc.gpsimd.dma_start(vb_bc[:], vb.ap().to_broadcast((128, 2 * D)))

            # ---------------- x loads (sync queue, chunked) ----------------
            x_view = x.ap().rearrange("(i p) c -> p i c", p=128)
            x_sb = singles.tile([128, NT, C], bf16)
            xT_view = xT.ap().rearrange("(cc p) t -> p cc t", p=128)
            xT_sb = singles.tile([128, CCH, T], bf16)
            for tch in range(TC):
                nc.sync.dma_start(
                    x_sb[:, 4 * tch : 4 * tch + 4, :],
                    x_view[:, 4 * tch : 4 * tch + 4, :],
                )
                nc.sync.dma_start(
                    xT_sb[:, :, 512 * tch : 512 * tch + 512],
                    xT_view[:, :, 512 * tch : 512 * tch + 512],
                )

            # ---------------- persistent tiles ----------------
            mv_sb = singles.tile([128, NT, 2], f32)
            eps_sb = singles.tile([128, 1], f32)
            nc.vector.memset(eps_sb[:], EPS)
            lnv = singles.tile([128, NT], f32)
            rstd = singles.tile([128, NT], f32)  # token-major 1/sqrt(var+eps)
            nmu = singles.tile([128, NT], f32)  # token-major -mu
            qT_sb = singles.tile([128, T], bf16)
            kT_sb = singles.tile([128, T], bf16)
            v_sb = singles.tile([128, NT, 2, 128], bf16)
            nc.gpsimd.memset(v_sb[:], 1.0)  # ones column at [.., D]
            attn_nm = singles.tile([D, 2, T], bf16)
            rstd_bc = singles.tile([128, T], f32)
            nmuT_sb = singles.tile([4, TC, 128], bf16)
            rstdT_sb = singles.tile([4, TC, 128], f32)
            from concourse.masks import make_identity
            ident_f32 = singles.tile([128, 128], f32)
            make_identity(nc, ident_f32[:])

            def emit_stats(tch):
                sl = slice(4 * tch, 4 * tch + 4)
                for i in range(4 * tch, 4 * tch + 4):
                    stats = work.tile([128, 6], f32, tag="bnstats")
                    nc.vector.bn_stats(out=stats[:], in_=x_sb[:, i, :])
                    nc.vector.bn_aggr(out=mv_sb[:, i, :], in_=stats[:])
                # rstd = rsqrt(var+eps): bit-hack seed + 2 Newton iters, all
                # on DVE back-to-back (cross-engine hops cost ~1-3us each in
                # queue+semaphore latency, so keep this chain on one engine)
                g = nc.vector
                xv = work.tile([128, 4], f32, tag="nt_x", bufs=2)
                g.tensor_scalar_add(xv[:], mv_sb[:, sl, 1], EPS)
                y = work.tile([128, 4], f32, tag="nt_y", bufs=2)
                yi = y.bitcast(i32)
                g.tensor_scalar(
                    out=yi[:], in0=xv.bitcast(i32)[:], scalar1=1,
                    scalar2=0xFFFFFFFF, op0=OP.logical_shift_right, op1=OP.bitwise_xor,
                )
                g.tensor_scalar_add(yi[:], yi[:], 0x5F3759DF + 1)
                t1 = work.tile([128, 4], f32, tag="nt_t", bufs=2)
                for it in range(2):
                    g.tensor_tensor(out=t1[:], in0=y[:], in1=y[:], op=OP.mult)
                    g.tensor_tensor(out=t1[:], in0=t1[:], in1=xv[:], op=OP.mult)
                    g.tensor_scalar(
                        out=t1[:], in0=t1[:], scalar1=-0.5, scalar2=1.5,
                        op0=OP.mult, op1=OP.add,
                    )
                    dst = rstd[:, sl] if it == 1 else y[:]
                    g.tensor_tensor(out=dst, in0=y[:], in1=t1[:], op=OP.mult)
                g.tensor_scalar_mul(nmu[:, sl], mv_sb[:, sl, 0], -1.0)
                # stat rows on-chip: transpose [128,4] token-major stats into
                # [4,128] row pieces (one partition per 128-token tile), then
                # build the rstd column-broadcast with K=1 outer matmuls
                ps_t = ps_proj.tile([128, 512], f32, tag="proj", name=f"ps_t{tch}")
                nc.tensor.transpose(ps_t[0:4, 0:128], nmu[:, sl], ident_f32[:])
                nc.tensor.transpose(ps_t[0:4, 128:256], rstd[:, sl], ident_f32[:])
                nc.vector.tensor_copy(out=nmuT_sb[:, tch, :], in_=ps_t[0:4, 0:128])
                nc.vector.tensor_copy(out=rstdT_sb[:, tch, :], in_=ps_t[0:4, 128:256])
                ps_bc = ps_proj.tile([128, 512], f32, tag="proj", name=f"ps_bc{tch}")
                for li in range(4):
                    nc.tensor.matmul(
                        ps_bc[:, 128 * li : 128 * li + 128],
                        lhsT=sel4_sb[:, li, :],
                        rhs=rstdT_sb[:, tch, :],
                        start=True,
                        stop=True,
                    )
                ts = slice(512 * tch, 512 * tch + 512)
                nc.vector.tensor_copy(out=rstd_bc[:, ts], in_=ps_bc[:])

            def emit_qk_proj(tch):
                ts = slice(512 * tch, 512 * tch + 512)
                ps_q = ps_proj.tile([128, 512], f32, tag="proj")
                for cc in range(CCH):
                    nc.tensor.matmul(
                        ps_q[:],
                        lhsT=wq_sb[:, cc, :],
                        rhs=xT_sb[:, cc, ts],
                        start=(cc == 0),
                        stop=False,
                    )
                for li in range(4):
                    nc.tensor.matmul(
                        ps_q[:, 128 * li : 128 * li + 128],
                        lhsT=csq_sb[:, li, :],
                        rhs=nmuT_sb[:, tch, :],
                        start=False,
                        stop=(li == 3),
                    )
                nc.vector.tensor_tensor(
                    out=qT_sb[:, ts], in0=ps_q[:], in1=rstd_bc[:, ts], op=OP.mult
                )
                if add_bias:
                    nc.vector.tensor_scalar(
                        out=qT_sb[:, ts], in0=qT_sb[:, ts], scalar1=qb_sb[:],
                        scalar2=None, op0=OP.add,
                    )
                ps_k = ps_proj.tile([128, 512], f32, tag="proj")
                for cc in range(CCH):
                    nc.tensor.matmul(
                        ps_k[:],
                        lhsT=wk_sb[:, cc, :],
                        rhs=xT_sb[:, cc, ts],
                        start=(cc == 0),
                        stop=False,
                    )
                for li in range(4):
                    nc.tensor.matmul(
                        ps_k[:, 128 * li : 128 * li + 128],
                        lhsT=csk_sb[:, li, :],
                        rhs=nmuT_sb[:, tch, :],
                        start=False,
                        stop=(li == 3),
                    )
                nc.vector.tensor_tensor(
                    out=kT_sb[:, ts], in0=ps_k[:], in1=rstd_bc[:, ts], op=OP.mult
                )
                if add_bias:
                    nc.gpsimd.tensor_scalar(
                        out=kT_sb[:, ts], in0=kT_sb[:, ts], scalar1=kb_sb[:],
                        scalar2=None, op0=OP.add,
                    )

            def emit_v_proj(tt):
                tsl = slice(128 * tt, 128 * tt + 128)
                ps_v = ps_proj.tile([128, 512], f32, tag="proj")
                for cc in range(CCH):
                    nc.tensor.matmul(
                        ps_v[:, 0:128],
                        lhsT=xT_sb[:, cc, tsl],
                        rhs=wv_sb[:, cc, :],
                        start=(cc == 0),
                        stop=False,
                    )
                nc.tensor.matmul(
                    ps_v[:, 0:128],
                    lhsT=nmuT_sb[:, tt // 4, :],
                    rhs=csv_sb[:, tt % 4, :],
                    start=False,
                    stop=True,
                )
                pv = ps_v[:, 0:128].rearrange("p (h d) -> p h d", h=2)
                if tt < 8:
                    # head chunks: ACT is idle before the first softmax exp
                    nc.scalar.mul(v_sb[:, tt, :, 0:D], pv, rstd[:, tt : tt + 1])
                else:
                    nc.vector.tensor_scalar(
                        out=v_sb[:, tt, :, 0:D], in0=pv,
                        scalar1=rstd[:, tt : tt + 1], scalar2=None, op0=OP.mult,
                    )
                if add_bias:
                    nc.gpsimd.tensor_tensor(
                        out=v_sb[:, tt, :, 0:D], in0=v_sb[:, tt, :, 0:D],
                        in1=vb_bc[:].rearrange("p (h d) -> p h d", h=2), op=OP.add,
                    )

            def emit_chunk(tch):
                emit_stats(tch)
                emit_qk_proj(tch)
                for tt in range(4 * tch, 4 * tch + 4):
                    emit_v_proj(tt)

            # ---------------- attention machinery ----------------
            av_tiles = {}

            def emit_event(half, kf, j, h, av):
                """scores + exp + AV for one key block in an (half, h) pass."""
                kb = 2 * kf + j
                c0 = 2 * half
                vis = [c for c in (c0, c0 + 1) if 2 * c + 1 >= kf]
                sc = ps_sc.tile([128, 2, 512], f32, tag="sc")
                e_t = epool.tile([128, 2, 512], bf16, tag="e")
                off = None
                for c in vis:
                    cr = c - c0
                    edge = (kf % 2 == 1) and (c == (kf - 1) // 2)
                    qoff = 256 if edge else 0
                    if edge:
                        off = cr * 512 + 256
                    nc.tensor.matmul(
                        sc[:, cr, qoff:512],
                        lhsT=kT_sb[64 * h : 64 * h + 64, 128 * kb : 128 * kb + 128],
                        rhs=qT_sb[64 * h : 64 * h + 64, 512 * c + qoff : 512 * c + 512],
                        start=True,
                        stop=True,
                    )
                if off is None:
                    off = (vis[0] - c0) * 512
                n = (vis[-1] - c0) * 512 + 512 - off
                sc_flat = sc.rearrange("p a q -> p (a q)")
                e_flat = e_t.rearrange("p a q -> p (a q)")
                nc.scalar.activation(
                    out=e_flat[:, off : off + n], in_=sc_flat[:, off : off + n],
                    func=AF.Exp,
                )
                # AV covers exactly the exp'd range of each chunk: no masking
                for c in vis:
                    cr = c - c0
                    qoff = off - cr * 512 if off > cr * 512 else 0
                    nc.tensor.matmul(
                        av[:, cr, qoff:512],
                        lhsT=v_sb[:, kb, h, :],
                        rhs=e_t[:, cr, qoff:512],
                        start=(kb == 0),
                        stop=(kb == 4 * c + 3),
                        skip_group_check=True,
                    )

            def emit_epilogue(c, h, av):
                """denominator reciprocal + normalize for (chunk, head)."""
                cr = c % 2
                den = epi.tile([D, 512], f32, tag="den")
                nc.vector.tensor_copy(out=den[:], in_=av[D : 2 * D, cr, :])
                rec = epi.tile([D, 512], f32, tag="rec")
                nc.vector.reciprocal_approx_fast(out=rec[:], in_=den[:])
                nc.vector.tensor_tensor(
                    out=attn_nm[:, h, 512 * c : 512 * c + 512],
                    in0=av[0:D, cr, :],
                    in1=rec[:],
                    op=OP.mult,
                )

            def emit_outproj(c, Cc):
                ts = slice(512 * c, 512 * c + 512)
                ps_o = ps_proj.tile([128, 512], f32, tag="proj")
                nc.tensor.matmul(
                    ps_o[:],
                    lhsT=wo_sb[:, 0, 128 * Cc : 128 * Cc + 128],
                    rhs=attn_nm[:, 0, ts],
                    start=True,
                    stop=False,
                )
                nc.tensor.matmul(
                    ps_o[:],
                    lhsT=wo_sb[:, 1, 128 * Cc : 128 * Cc + 128],
                    rhs=attn_nm[:, 1, ts],
                    start=False,
                    stop=True,
                )
                stage = work.tile([128, 512], bf16, tag="ostage", bufs=4)
                nc.vector.tensor_copy(out=stage[:], in_=ps_o[:])
                nc.sync.dma_start(out.ap()[128 * Cc : 128 * Cc + 128, ts], stage[:])

            # ---------------- emission schedule ----------------
            emit_chunk(0)
            emit_chunk(1)

            # four passes: (half, h); av accumulators are one 2-bank pair
            fillers = []
            for tch in (2, 3):
                fillers.append(lambda tch=tch: emit_stats(tch))
                fillers.append(lambda tch=tch: emit_qk_proj(tch))
                for tt in range(4 * tch, 4 * tch + 4):
                    fillers.append(lambda tt=tt: emit_v_proj(tt))
            fi = 0
            for half in (0, 1):
                if half == 1:
                    fillers = [
                        (lambda c=c, Cc=Cc: emit_outproj(c, Cc))
                        for c in (0, 1)
                        for Cc in range(4)
                    ]
                    fi = 0
                for h in (0, 1):
                    av = ps_av.tile(
                        [128, 2, 512], f32, tag="av", name=f"av_{half}_{h}"
                    )
                    for kf in range(4 * (half + 1)):
                        for j in range(2):
                            emit_event(half, kf, j, h, av)
                        if fi < len(fillers):
                            fillers[fi]()
                            fi += 1
                    for c in (2 * half, 2 * half + 1):
                        emit_epilogue(c, h, av)
                while fi < len(fillers):
                    fillers[fi]()
                    fi += 1
            for c in (2, 3):
                for Cc in range(4):
                    emit_outproj(c, Cc)

    nc.compile()
    return nc


def _sel_rows(v):
    # [4, 4*128]: block li holds v on row k==li, zeros elsewhere
    out = np.zeros((4, 4 * 128), dtype=np.float32)
    for li in range(4):
        out[li, 128 * li : 128 * li + 128] = v
    return np.ascontiguousarray(out)


def _make_in_maps(x, ln_gamma, ln_beta, w_qkv, w_out, b_out):
    bf = ml_dtypes.bfloat16
    x = np.asarray(x, dtype=np.float32)
    g = np.asarray(ln_gamma, dtype=np.float32)
    beta = np.asarray(ln_beta, dtype=np.float32)
    w_qkv = np.asarray(w_qkv, dtype=np.float32)
    w_out = np.asarray(w_out, dtype=np.float32)
    b_out = np.asarray(b_out, dtype=np.float32)

    wg = w_qkv * g[:, None]  # fold gamma into the projection
    bias_qkv = beta @ w_qkv  # fold beta into additive biases
    add_bias = bool(np.any(bias_qkv != 0))
    scale = D ** -0.5

    in_maps = []
    for c in range(N_CORES):
        b, hp = c % 2, c // 2
        qs = slice(128 * hp, 128 * hp + 128)
        ks = slice(INNER + 128 * hp, INNER + 128 * hp + 128)
        vs = slice(2 * INNER + 128 * hp, 2 * INNER + 128 * hp + 128)
        wq_c = wg[:, qs] * scale
        wk_c = wg[:, ks]
        wv_c = wg[:, vs]
        m = {
            "x": np.ascontiguousarray(x[b].astype(bf)),
            "xT": np.ascontiguousarray(x[b].T.astype(bf)),
            "wq": np.ascontiguousarray(wq_c.astype(bf)),
            "wk": np.ascontiguousarray(wk_c.astype(bf)),
            "wv": np.ascontiguousarray(wv_c.astype(bf)),
            "csq": _sel_rows(wq_c.sum(axis=0)).astype(bf),
            "csk": _sel_rows(wk_c.sum(axis=0)).astype(bf),
            "csv": _sel_rows(wv_c.sum(axis=0)).astype(bf),
            "sel4": _sel_rows(np.ones(128, dtype=np.float32)).astype(np.float32),
            "w_out": np.ascontiguousarray(w_out[128 * hp : 128 * (hp + 1), :].astype(bf)),
        }
        if add_bias:
            m["qb"] = np.ascontiguousarray(
                (bias_qkv[qs] * scale).reshape(128, 1).astype(np.float32)
            )
            m["kb"] = np.ascontiguousarray(bias_qkv[ks].reshape(128, 1).astype(np.float32))
            m["vb"] = np.ascontiguousarray(bias_qkv[vs].reshape(1, 128).astype(np.float32))
        in_maps.append(m)
    return in_maps, add_bias, b_out


def _run(inputs, trace=False, trace_cores=None):
    from concourse.bass_utils import run_bass_kernel_spmd

    in_maps, add_bias, b_out = _make_in_maps(**inputs)
    key = ("nc", add_bias)
    if key not in _CACHE:
        _CACHE[key] = _build(add_bias)
    nc = _CACHE[key]
    res = run_bass_kernel_spmd(
        nc,
        in_maps,
        core_ids=list(range(N_CORES)),
        trace=trace,
        trace_cores=trace_cores,
    )
    # sum-unshard: out-projection is row-parallel across head-pairs; each core
    # returns an outT [C, T] bf16 partial; host transposes, sums, adds bias.
    full = np.zeros((B, T, C), dtype=np.float32)
    for c in range(N_CORES):
        full[c % 2] += res.results[c]["out"].astype(np.float32).T
    full += b_out.reshape(1, 1, C)
    return full, res


def kernel(**inputs):
    full, _ = _run(inputs, trace=False)
    return full
